# revision 1
# baseline (speedup 1.0000x reference)
"""Trainium2 Bass kernel for nn_BiLSTM_58351425683848.

Self-contained: accepts the FULL inputs of reference.setup_inputs(), returns
the FULL [256, 1024] output. Internally row-shards the sequence dim across 8
NeuronCores (the only cross-core data each step is the 16KB of BatchNorm
statistics, exchanged with two 8KB AllGathers); all GEMMs run in fp16 hi/lo
split arithmetic (fp32-equivalent accuracy at ~4x fp32 PE throughput).

Only the live part of the reference is computed: the LSTM cell updates, W4/b4
and the per-step outputs are dead code — the result is out[-1] =
0.5*(hf2+hb2) at t=255 of the interaction/BatchNorm recurrence.
"""
import sys
sys.path.insert(0, '/opt/trn_rl_repo')
import numpy as np

S = 256
H = 1024
EPS = 1e-5
NK = 8
SL = 32


def pack_actT(a):
    sl = a.shape[0]
    out = np.empty((128, NK * sl), a.dtype)
    for k in range(NK):
        out[:, k * sl:(k + 1) * sl] = a[:, k * 128:(k + 1) * 128].T
    return np.ascontiguousarray(out)


def unpack_actT(p, sl=SL):
    a = np.empty((sl, H), p.dtype)
    for k in range(NK):
        a[:, k * 128:(k + 1) * 128] = p[:, k * sl:(k + 1) * sl].T
    return a


def pack_w_moving(w):
    out = np.empty((128, NK * H), w.dtype)
    for k in range(NK):
        out[:, k * H:(k + 1) * H] = w[:, k * 128:(k + 1) * 128].T
    return np.ascontiguousarray(out)


def pack_vec(v):
    return np.ascontiguousarray(v.reshape(NK, 128).T)


def split16(x):
    hi = x.astype(np.float16)
    lo = (x - hi.astype(np.float32)).astype(np.float16)
    return hi, lo


def build_kernel(nsteps, n_cores=8, debug_taps=()):
    import sys
    sys.path.insert(0, '/opt/trn_rl_repo')
    import concourse.bacc as bacc
    import concourse.tile as tile
    import concourse.mybir as mybir

    f32 = mybir.dt.float32
    f16 = mybir.dt.float16
    AFT = mybir.ActivationFunctionType
    ALU = mybir.AluOpType

    nc = bacc.Bacc("TRN2", target_bir_lowering=False, debug=False,
                   num_devices=n_cores)

    xt = nc.dram_tensor("xt", [128, NK * SL], f32, kind="ExternalInput")
    w_in = {}
    for nm in ("w1h", "w1l", "w2h", "w2l", "w3h", "w3l"):
        w_in[nm] = nc.dram_tensor(nm, [128, NK * H], f16, kind="ExternalInput")
    # bias rows: (b_hi; b_lo) [2, 1024] per gemm -> packed [2, 3*1024] fp16
    brow_in = nc.dram_tensor("brow", [2, 3 * H], f16, kind="ExternalInput")
    vecs_in = nc.dram_tensor("vecs", [128, NK * 6], f32, kind="ExternalInput")
    # vecs: gf bf gb bb (4 used of 6)
    outp = nc.dram_tensor("out", [128, NK * SL], f32, kind="ExternalOutput")
    taps = {}
    for nm in debug_taps:
        taps[nm] = nc.dram_tensor(f"tap_{nm}", [128, NK * SL], f32,
                                  kind="ExternalOutput")

    E_np = np.tile(np.eye(SL, dtype=np.float32), (4, 1))
    e_dram = nc.inline_tensor(E_np, name="emat")
    ones2_np = np.ones((2, SL), dtype=np.float16)
    ones2_dram = nc.inline_tensor(ones2_np, name="ones2")

    with tile.TileContext(nc) as tc:
        with tc.tile_pool(name="wpool", bufs=1) as wpool, \
             tc.tile_pool(name="spool", bufs=3) as spool, \
             tc.tile_pool(name="dpool", bufs=4, space="DRAM") as dpool, \
             tc.tile_pool(name="ppool", bufs=2, space="PSUM") as ppool, \
             tc.tile_pool(name="pxpool", bufs=2, space="PSUM") as pxpool, \
             tc.tile_pool(name="warmp", bufs=1, space="PSUM") as warmp:

            w_sb = {}
            for nm in w_in:
                w_sb[nm] = wpool.tile([128, NK * H], f16, tag=nm, name=nm)
                for k in range(NK):
                    nc.sync.dma_start(w_sb[nm][:, k * H:(k + 1) * H],
                                      w_in[nm][:, k * H:(k + 1) * H])
            brow = wpool.tile([2, 3 * H], f16, tag="brow")
            nc.sync.dma_start(brow[:], brow_in[:])
            vecs = wpool.tile([128, NK * 6], f32, tag="vecs")
            nc.sync.dma_start(vecs[:], vecs_in[:])
            gfp = vecs[:, 0 * NK:1 * NK]
            bfp = vecs[:, 1 * NK:2 * NK]
            gbp = vecs[:, 2 * NK:3 * NK]
            bbp = vecs[:, 3 * NK:4 * NK]
            e_sb = wpool.tile([128, SL], f32, tag="emat")
            nc.sync.dma_start(e_sb[:], e_dram[:])
            ones2 = wpool.tile([2, SL], f16, tag="ones2")
            nc.sync.dma_start(ones2[:], ones2_dram[:])
            xt_sb = wpool.tile([128, NK * SL], f32, tag="xt")
            nc.sync.dma_start(xt_sb[:], xt[:])
            epsc = wpool.tile([128, 1], f32, tag="epsc")
            nc.vector.memset(epsc[:], EPS)

            hfT = wpool.tile([128, NK * SL], f32, tag="hfT")
            hbT = wpool.tile([128, NK * SL], f32, tag="hbT")
            nc.vector.memset(hfT[:], 0.0)
            nc.vector.memset(hbT[:], 0.0)

            PW = NK * SL

            def split_a(a, tagbase):
                ah = spool.tile([128, PW], f16, tag=tagbase + "h",
                                name=tagbase + "h")
                al = spool.tile([128, PW], f16, tag=tagbase + "l",
                                name=tagbase + "l")
                nc.vector.tensor_copy(ah[:], a[:])
                nc.vector.tensor_sub(al[:], a[:], ah[:])
                return ah, al

            def add_split(x, y, tagbase):
                """(ah, al) = fp16 hi/lo of (x + y), no fp32 materialization."""
                ah = spool.tile([128, PW], f16, tag=tagbase + "h",
                                name=tagbase + "h")
                al = spool.tile([128, PW], f16, tag=tagbase + "l",
                                name=tagbase + "l")
                tr = spool.tile([128, PW], f32, tag="addres", name="addres")
                nc.vector.tensor_add(ah[:], x[:], y[:])
                nc.vector.tensor_sub(tr[:], x[:], ah[:])
                nc.vector.tensor_add(al[:], tr[:], y[:])
                return ah, al

            def gemm(ah, al, wh, wl, bslice, px, copy_engine):
                """PSUM <- Ah@Wh + Ah@Wl + Al@Wh + bias; E-matmul into px."""
                P = ppool.tile([128, H], f32, tag="P", name="P")
                units = [(k, p) for k in range(NK) for p in range(3)]
                per_group = [[] for _ in range(4)]
                for ui, u in enumerate(units):
                    per_group[ui % 4].append(u)
                per_group[0].insert(0, "bias")
                for g in range(4):
                    lst = per_group[g]
                    for idx, u in enumerate(lst):
                        first = (idx == 0)
                        last_u = (idx == len(lst) - 1)
                        if u == "bias":
                            lhs_ap = ones2[:]
                            w_ap = lambda nh: bslice[:, 512 * nh:512 * (nh + 1)]
                        else:
                            k, p = u
                            lhs = (ah if p in (0, 1) else al)
                            w = (w_sb[wh] if p in (0, 2) else w_sb[wl])
                            lhs_ap = lhs[:, k * SL:(k + 1) * SL]
                            w_ap = (lambda nh, w=w, k=k:
                                    w[:, k * H + 512 * nh:k * H + 512 * (nh + 1)])
                        for nh in range(2):
                            nc.tensor.matmul(
                                P[32 * g:32 * (g + 1), 512 * nh:512 * (nh + 1)],
                                lhs_ap,
                                w_ap(nh) if callable(w_ap) else w_ap,
                                start=first, stop=last_u,
                                tile_position=(0, 32 * g),
                            )
                Ssb = spool.tile([128, H], f32, tag="Ssb", name="Ssb")
                if copy_engine == "act":
                    nc.scalar.activation(Ssb[:], P[:], AFT.Copy)
                else:
                    nc.vector.tensor_copy(Ssb[:], P[:])
                for j in range(NK):
                    nc.tensor.matmul(
                        px[:, j * SL:(j + 1) * SL],
                        Ssb[:, j * 128:(j + 1) * 128],
                        e_sb[:],
                        start=True, stop=True,
                    )
                return px

            def stats_of(hx2, tag):
                # returns [128,16] tile: cols 0:8 sums, 8:16 sumsq
                st = spool.tile([128, 16], f32, tag="st" + tag, name="st" + tag)
                nc.vector.tensor_reduce(st[:, 0:8],
                                        hx2[:].rearrange("p (j s) -> p j s", j=NK),
                                        axis=mybir.AxisListType.X, op=ALU.add)
                sq_ = spool.tile([128, PW], f32, tag="sqscr", name="sq" + tag)
                nc.scalar.activation(sq_[:], hx2[:], AFT.Square)
                nc.vector.tensor_reduce(st[:, 8:16],
                                        sq_[:].rearrange("p (j s) -> p j s", j=NK),
                                        axis=mybir.AxisListType.X, op=ALU.add)
                return st

            def launch_ag(st, tag):
                inb = dpool.tile([128, 16], f32, tag="agi" + tag,
                                 name="agi" + tag)
                outb = dpool.tile([128 * n_cores, 16], f32, tag="ago" + tag,
                                  name="ago" + tag)
                nc.sync.dma_start(inb[:], st[:])
                nc.gpsimd.collective_compute(
                    "AllGather", ALU.bypass,
                    replica_groups=[list(range(n_cores))],
                    ins=[inb.opt()], outs=[outb.opt()],
                )
                return outb

            def bn_apply(outb, gamma, beta, hx2, hxT, tag):
                """Gather -> totals -> params -> hxT = a*hx2 + c."""
                gath = spool.tile([128, n_cores * 16], f32, tag="gath" + tag,
                                  name="gath" + tag)
                nc.sync.dma_start(
                    gath[:].rearrange("p (r c) -> p r c", r=n_cores),
                    outb[:].rearrange("(r p) c -> p r c", p=128))
                tot = spool.tile([128, 16], f32, tag="tot" + tag,
                                 name="tot" + tag)
                nc.vector.tensor_reduce(
                    tot[:], gath[:].rearrange("p (r c) -> p c r", r=n_cores),
                    axis=mybir.AxisListType.X, op=ALU.add)
                prm = spool.tile([128, 40], f32, tag="prm" + tag,
                                 name="prm" + tag)
                mean = prm[:, 0:8]
                var = prm[:, 8:16]
                a_ = prm[:, 16:24]
                c_ = prm[:, 24:32]
                msq = prm[:, 32:40]
                nc.vector.tensor_scalar_mul(mean, tot[:, 0:8], 1.0 / S)
                nc.vector.tensor_mul(msq, mean, mean)
                nc.vector.tensor_scalar(var, tot[:, 8:16], 1.0 / S, None,
                                        ALU.mult)
                nc.vector.tensor_sub(var, var, msq)
                sq = spool.tile([128, 8], f32, tag="sqv" + tag,
                                name="sqv" + tag)
                nc.scalar.activation(sq[:], var, AFT.Sqrt, bias=epsc[:, 0:1])
                r0 = spool.tile([128, 8], f32, tag="r0" + tag,
                                name="r0" + tag)
                nc.vector.reciprocal(r0[:], sq[:])
                nr = spool.tile([128, 24], f32, tag="nr" + tag,
                                name="nr" + tag)
                nc.vector.tensor_mul(nr[:, 0:8], r0[:], r0[:])
                nc.vector.tensor_scalar(nr[:, 8:16], var, EPS, -0.5,
                                        ALU.add, ALU.mult)
                nc.vector.tensor_mul(nr[:, 0:8], nr[:, 0:8], nr[:, 8:16])
                nc.vector.tensor_scalar_add(nr[:, 0:8], nr[:, 0:8], 1.5)
                nc.vector.tensor_mul(r0[:], r0[:], nr[:, 0:8])
                nc.vector.tensor_mul(a_, gamma, r0[:])
                nc.vector.tensor_mul(c_, a_, mean)
                nc.vector.tensor_sub(c_, beta, c_)
                for j in range(NK):
                    nc.vector.tensor_scalar(
                        hxT[:, j * SL:(j + 1) * SL],
                        hx2[:, j * SL:(j + 1) * SL],
                        a_[:, j:j + 1], c_[:, j:j + 1],
                        ALU.mult, ALU.add)

            # ---- pipelined main loop ----
            # carried across iterations: pending AG_b + hb2 of previous step
            pend_b = None  # (outb, hb2_tile)
            for t in range(nsteps):
                last = (t == nsteps - 1)
                # G1 (PE busy while pending AG_b is in flight)
                a1h, a1l = add_split(xt_sb, hfT, "a1s")
                px1 = pxpool.tile([128, PW], f32, tag="px", name="px1")
                gemm(a1h, a1l, "w1h", "w1l", brow[:, 0:H], px1, "act")
                x1 = spool.tile([128, PW], f32, tag="x1")
                nc.scalar.activation(x1[:], px1[:], AFT.Sigmoid)

                # finish previous step's backward BN (overlaps G1's MMs)
                if pend_b is not None:
                    outb_b, hb2_prev = pend_b
                    bn_apply(outb_b, gbp, bbp, hb2_prev, hbT, "b")
                    pend_b = None

                # G3 (forward)
                a3h, a3l = add_split(x1, hfT, "a3s")
                px3 = pxpool.tile([128, PW], f32, tag="px", name="px3")
                gemm(a3h, a3l, "w3h", "w3l", brow[:, 2 * H:3 * H], px3, "act")
                hf2 = spool.tile([128, PW], f32, tag="hf2")
                nc.scalar.activation(hf2[:], px3[:], AFT.Sigmoid)
                if not last:
                    st_f = stats_of(hf2, "f")
                    outb_f = launch_ag(st_f, "f")

                # G2 (backward) — PE busy while AG_f in flight
                a2h, a2l = add_split(hbT, x1, "a2s")
                px2 = pxpool.tile([128, PW], f32, tag="px", name="px2")
                gemm(a2h, a2l, "w2h", "w2l", brow[:, H:2 * H], px2, "act")
                hb2 = spool.tile([128, PW], f32, tag="hb2")
                nc.scalar.activation(hb2[:], px2[:], AFT.Sigmoid)

                if last:
                    o = spool.tile([128, PW], f32, tag="o")
                    nc.vector.tensor_add(o[:], hf2[:], hb2[:])
                    nc.vector.tensor_scalar_mul(o[:], o[:], 0.5)
                    nc.sync.dma_start(outp[:], o[:])
                    for nm, ap in (("x1", x1), ("hf2", hf2), ("hb2", hb2)):
                        if nm in taps:
                            nc.sync.dma_start(taps[nm][:], ap[:])
                    continue

                st_b = stats_of(hb2, "b")
                outb_b = launch_ag(st_b, "b")
                pend_b = (outb_b, hb2)

                # keep-warm: dummy MMs anchored on a2h fill the AG_f wait so
                # the PE's HAM clock gate stays at 2.4GHz across the gap
                wp = warmp.tile([128, 512], f32, tag="wp", name="wp")
                for d in range(8):
                    nc.tensor.matmul(wp[0:32, :],
                                     a2h[:, (d % NK) * SL:((d % NK) + 1) * SL],
                                     w_sb["w1h"][:, 0:512],
                                     start=True, stop=True,
                                     skip_group_check=True)
                wscr = spool.tile([128, 8], f32, tag="wscr", name="wscr")
                nc.vector.tensor_copy(wscr[:32, :], wp[0:32, 0:8])

                # forward BN for next step's G1 (AG_f should have landed)
                bn_apply(outb_f, gfp, bfp, hf2, hfT, "f")

    nc.compile()
    return nc


def numpy_sim(inp, nsteps):
    sig = lambda x: 1.0 / (1.0 + np.exp(-x))

    def bn(x, g, b):
        m = x.mean(0)
        xc = x - m
        v = (xc * xc).mean(0)
        return xc / np.sqrt(v + EPS) * g + b

    X = inp["inputs"]
    hf = np.zeros((S, H), np.float32)
    hb = np.zeros((S, H), np.float32)
    for t in range(nsteps):
        x1 = sig((X + hf) @ inp["W1"].T + inp["b1"])
        hb2 = sig((hb + x1) @ inp["W2"].T + inp["b2"])
        hf2 = sig((x1 + hf) @ inp["W3"].T + inp["b3"])
        out = (hf2 + hb2) * 0.5
        hf = bn(hf2, inp["gamma_f"], inp["beta_f"])
        hb = bn(hb2, inp["gamma_b"], inp["beta_b"])
    return out, x1, hf2, hb2


def make_in_maps(inp, n_cores=8):
    m = {}
    for i, wn in enumerate(("W1", "W2", "W3")):
        wh, wl = split16(np.asarray(inp[wn], np.float32))
        m[f"w{i+1}h"] = pack_w_moving(wh)
        m[f"w{i+1}l"] = pack_w_moving(wl)
    brow = np.zeros((2, 3 * H), np.float16)
    for i, bn_ in enumerate(("b1", "b2", "b3")):
        bh, bl = split16(np.asarray(inp[bn_], np.float32))
        brow[0, i * H:(i + 1) * H] = bh
        brow[1, i * H:(i + 1) * H] = bl
    m["brow"] = brow
    vecs = np.zeros((128, NK * 6), np.float32)
    for i, nm in enumerate(("gamma_f", "beta_f", "gamma_b", "beta_b")):
        vecs[:, i * NK:(i + 1) * NK] = pack_vec(np.asarray(inp[nm], np.float32))
    m["vecs"] = vecs
    X = np.asarray(inp["inputs"], np.float32)
    maps = []
    for c in range(n_cores):
        mm = dict(m)
        mm["xt"] = pack_actT(X[c * SL:(c + 1) * SL, :])
        maps.append(mm)
    return maps


def assemble_out(results, n_cores=8):
    out = np.empty((S, H), np.float32)
    for c in range(n_cores):
        out[c * SL:(c + 1) * SL, :] = unpack_actT(results[c]["out"])
    return out


_NC_CACHE = {}


def kernel(**inputs):
    import numpy as np
    nsteps = S  # 256 scan steps
    key = nsteps
    if key not in _NC_CACHE:
        _NC_CACHE[key] = build_kernel(nsteps)
    nc = _NC_CACHE[key]
    inp = {k: np.asarray(v) for k, v in inputs.items()}
    maps = make_in_maps(inp)
    from concourse.bass_utils import run_bass_kernel_spmd
    res = run_bass_kernel_spmd(nc, maps, core_ids=list(range(8)))
    return assemble_out(res.results).astype(np.float32)



# revision 2
# speedup vs baseline: 1.0037x; 1.0037x over previous
"""Trainium2 Bass kernel v3 for nn_BiLSTM_58351425683848.

Self-contained: accepts FULL inputs of reference.setup_inputs(), returns the
FULL [256, 1024] output. Row-shards the sequence across 8 cores (32 rows
each); per-step BN statistics are exchanged with two staggered 8KB
AllGathers.

Arithmetic: fp16 hi/lo 3-product gemms (fp32-equivalent), but with packed
stationaries so each gemm streams each weight matrix only twice instead of
three times: the Wh stream uses stationary [uh|ul] (M=64, col strips 0-1)
and the Wl stream uses [uh] (M=32, strips 2-3, alternating by k); the
existing 4-block E-matmul reduction sums uh@Wh + ul@Wh + uh@Wl for free.
"""
import sys
sys.path.insert(0, '/opt/trn_rl_repo')
import numpy as np

S = 256
H = 1024
EPS = 1e-5
NK = 8
SL = 32
PW = NK * SL  # 256


def pack_actT(a):
    sl = a.shape[0]
    out = np.empty((128, NK * sl), a.dtype)
    for k in range(NK):
        out[:, k * sl:(k + 1) * sl] = a[:, k * 128:(k + 1) * 128].T
    return np.ascontiguousarray(out)


def unpack_actT(p, sl=SL):
    a = np.empty((sl, H), p.dtype)
    for k in range(NK):
        a[:, k * 128:(k + 1) * 128] = p[:, k * sl:(k + 1) * sl].T
    return a


def pack_w_moving(w):
    out = np.empty((128, NK * H), w.dtype)
    for k in range(NK):
        out[:, k * H:(k + 1) * H] = w[:, k * 128:(k + 1) * 128].T
    return np.ascontiguousarray(out)


def pack_vec(v):
    return np.ascontiguousarray(v.reshape(NK, 128).T)


def split16(x):
    hi = x.astype(np.float16)
    lo = (x - hi.astype(np.float32)).astype(np.float16)
    return hi, lo


def build_kernel(nsteps, n_cores=8, use_bias=False, debug_taps=()):
    import concourse.bacc as bacc
    import concourse.tile as tile
    import concourse.mybir as mybir

    f32 = mybir.dt.float32
    f16 = mybir.dt.float16
    AFT = mybir.ActivationFunctionType
    ALU = mybir.AluOpType

    nc = bacc.Bacc("TRN2", target_bir_lowering=False, debug=False,
                   num_devices=n_cores)

    xt = nc.dram_tensor("xt", [128, PW], f32, kind="ExternalInput")
    w_in = {}
    for nm in ("w1h", "w1l", "w2h", "w2l", "w3h", "w3l"):
        w_in[nm] = nc.dram_tensor(nm, [128, NK * H], f16, kind="ExternalInput")
    # vecs: gf bf gb bb b1 b2 b3 (packed [128, NK] each)
    vecs_in = nc.dram_tensor("vecs", [128, NK * 7], f32, kind="ExternalInput")
    outp = nc.dram_tensor("out", [128, PW], f32, kind="ExternalOutput")
    taps = {}
    for nm in debug_taps:
        taps[nm] = nc.dram_tensor(f"tap_{nm}", [128, PW], f32,
                                  kind="ExternalOutput")

    E_np = np.tile(np.eye(SL, dtype=np.float32), (4, 1))
    e_dram = nc.inline_tensor(E_np, name="emat")

    with tile.TileContext(nc) as tc:
        with tc.tile_pool(name="wpool", bufs=1) as wpool, \
             tc.tile_pool(name="spool", bufs=3) as spool, \
             tc.tile_pool(name="dpool", bufs=4, space="DRAM") as dpool, \
             tc.tile_pool(name="ppool", bufs=2, space="PSUM") as ppool, \
             tc.tile_pool(name="pxpool", bufs=2, space="PSUM") as pxpool, \
             tc.tile_pool(name="warmp", bufs=1, space="PSUM") as warmp:

            w_sb = {}
            for nm in w_in:
                w_sb[nm] = wpool.tile([128, NK * H], f16, tag=nm, name=nm)
                for k in range(NK):
                    nc.sync.dma_start(w_sb[nm][:, k * H:(k + 1) * H],
                                      w_in[nm][:, k * H:(k + 1) * H])
            vecs = wpool.tile([128, NK * 7], f32, tag="vecs")
            nc.sync.dma_start(vecs[:], vecs_in[:])
            gfp = vecs[:, 0 * NK:1 * NK]
            bfp = vecs[:, 1 * NK:2 * NK]
            gbp = vecs[:, 2 * NK:3 * NK]
            bbp = vecs[:, 3 * NK:4 * NK]
            bias_g = {"1": vecs[:, 4 * NK:5 * NK],
                      "2": vecs[:, 5 * NK:6 * NK],
                      "3": vecs[:, 6 * NK:7 * NK]}
            e_sb = wpool.tile([128, SL], f32, tag="emat")
            nc.sync.dma_start(e_sb[:], e_dram[:])
            xt_sb = wpool.tile([128, PW], f32, tag="xt")
            nc.sync.dma_start(xt_sb[:], xt[:])

            hfT = wpool.tile([128, PW], f32, tag="hfT")
            hbT = wpool.tile([128, PW], f32, tag="hbT")
            nc.vector.memset(hfT[:], 0.0)
            nc.vector.memset(hbT[:], 0.0)
            # packed hi/lo stationaries, per chunk k: [uh_k (32) | ul_k (32)]
            u1 = wpool.tile([128, 2 * PW], f16, tag="u1")
            u3 = wpool.tile([128, 2 * PW], f16, tag="u3")
            u2 = wpool.tile([128, 2 * PW], f16, tag="u2")

            def uh_view(u):
                return u[:].rearrange("p (k c) -> p k c", k=NK)[:, :, 0:SL]

            def ul_view(u):
                return u[:].rearrange("p (k c) -> p k c", k=NK)[:, :, SL:2 * SL]

            def v3(x):
                return x[:].rearrange("p (k s) -> p k s", k=NK)

            # init u1 = split(xt + 0)
            nc.vector.tensor_copy(uh_view(u1), v3(xt_sb))
            nc.vector.tensor_sub(ul_view(u1), v3(xt_sb), uh_view(u1))

            def build_u_full(u, a32, b32):
                """u <- hi/lo split of (a32 + b32), full-tile ops."""
                t = spool.tile([128, PW], f32, tag="uscr", name="uscr")
                nc.vector.tensor_add(t[:], a32[:], b32[:])
                nc.vector.tensor_copy(uh_view(u), v3(t))
                nc.vector.tensor_sub(ul_view(u), v3(t), uh_view(u))

            def build_u_chunked(u, a32, b32):
                for k in range(NK):
                    sl_ = slice(k * SL, (k + 1) * SL)
                    t = spool.tile([128, SL], f32, tag="ucs", name=f"ucs{k}")
                    nc.vector.tensor_add(t[:], a32[:, sl_], b32[:, sl_])
                    nc.vector.tensor_copy(u[:, 64 * k:64 * k + SL], t[:])
                    nc.vector.tensor_sub(u[:, 64 * k + SL:64 * k + 64],
                                         t[:], u[:, 64 * k:64 * k + SL])

            def gemm(u, wh, wl, P):
                """P[128,1024] <- 3-product fp16 gemm with packed stationaries.

                Adjacent MMs alternate BOTH col strip and PSUM bank so their
                fill/drain phases overlap (pairing needs distinct banks)."""
                whs, wls = w_sb[wh], w_sb[wl]
                for k in range(NK):
                    for nh in range(2):
                        nc.tensor.matmul(
                            P[0:64, 512 * nh:512 * (nh + 1)],
                            u[:, 64 * k:64 * k + 64],
                            whs[:, k * H + 512 * nh:k * H + 512 * (nh + 1)],
                            start=(k == 0), stop=(k == NK - 1),
                            tile_position=(0, 0),
                        )
                for k in range(NK):
                    p = k % 2
                    for nh in range(2):
                        nc.tensor.matmul(
                            P[64 + 32 * p:96 + 32 * p, 512 * nh:512 * (nh + 1)],
                            u[:, 64 * k:64 * k + SL],
                            wls[:, k * H + 512 * nh:k * H + 512 * (nh + 1)],
                            start=(k == p), stop=(k == NK - 2 + p),
                            tile_position=(0, 64 + 32 * p),
                        )

            def reduce_transpose(P, tag):
                Ssb = spool.tile([128, H], f32, tag="Ssb", name="Ssb" + tag)
                nc.scalar.activation(Ssb[:, 0:512], P[:, 0:512], AFT.Copy)
                nc.vector.tensor_copy(Ssb[:, 512:1024], P[:, 512:1024])
                px = pxpool.tile([128, PW], f32, tag="px", name="px" + tag)
                for j in range(NK):
                    nc.tensor.matmul(
                        px[:, j * SL:(j + 1) * SL],
                        Ssb[:, j * 128:(j + 1) * 128],
                        e_sb[:],
                        start=True, stop=True,
                    )
                return px

            def sigmoid_into(px, dst, which):
                if use_bias:
                    for k in range(NK):
                        nc.scalar.activation(
                            dst[:, k * SL:(k + 1) * SL],
                            px[:, k * SL:(k + 1) * SL], AFT.Sigmoid,
                            bias=bias_g[which][:, k:k + 1])
                else:
                    nc.scalar.activation(dst[:], px[:], AFT.Sigmoid)

            def stats_of(hx2, tag):
                st = spool.tile([128, 16], f32, tag="st" + tag, name="st" + tag)
                nc.vector.tensor_reduce(st[:, 0:8], v3(hx2),
                                        axis=mybir.AxisListType.X, op=ALU.add)
                sq_ = spool.tile([128, PW], f32, tag="sqscr", name="sq" + tag)
                nc.scalar.activation(sq_[:], hx2[:], AFT.Square)
                nc.vector.tensor_reduce(st[:, 8:16], v3(sq_),
                                        axis=mybir.AxisListType.X, op=ALU.add)
                return st

            def launch_ag(st, tag):
                inb = dpool.tile([128, 16], f32, tag="agi" + tag,
                                 name="agi" + tag)
                outb = dpool.tile([128 * n_cores, 16], f32, tag="ago" + tag,
                                  name="ago" + tag)
                nc.sync.dma_start(inb[:], st[:])
                nc.gpsimd.collective_compute(
                    "AllGather", ALU.bypass,
                    replica_groups=[list(range(n_cores))],
                    ins=[inb.opt()], outs=[outb.opt()],
                )
                return outb

            _gath_tiles = {}

            def bn_params(outb, gamma, beta, tag):
                gath = spool.tile([128, n_cores * 16], f32, tag="gath" + tag,
                                  name="gath" + tag)
                _gath_tiles[tag] = gath
                nc.sync.dma_start(
                    gath[:].rearrange("p (r c) -> p r c", r=n_cores),
                    outb[:].rearrange("(r p) c -> p r c", p=128))
                tot = spool.tile([128, 16], f32, tag="tot" + tag,
                                 name="tot" + tag)
                nc.vector.tensor_reduce(
                    tot[:], gath[:].rearrange("p (r c) -> p c r", r=n_cores),
                    axis=mybir.AxisListType.X, op=ALU.add)
                prm = spool.tile([128, 48], f32, tag="prm" + tag,
                                 name="prm" + tag)
                a_ = prm[:, 0:8]
                c_ = prm[:, 8:16]
                mean = prm[:, 16:24]
                vpe = prm[:, 24:32]
                u_ = prm[:, 32:40]
                r0 = prm[:, 40:48]
                nc.vector.tensor_scalar_mul(mean, tot[:, 0:8], 1.0 / S)
                nc.vector.tensor_mul(u_, mean, mean)
                nc.vector.tensor_scalar(vpe, tot[:, 8:16], 1.0 / S, EPS,
                                        ALU.mult, ALU.add)
                nc.vector.tensor_sub(vpe, vpe, u_)  # var + eps
                sq = spool.tile([128, 8], f32, tag="sqv" + tag,
                                name="sqv" + tag)
                nc.scalar.activation(sq[:], vpe, AFT.Sqrt)
                nc.vector.reciprocal(r0, sq[:])
                nc.vector.tensor_mul(u_, r0, r0)
                nc.vector.tensor_mul(u_, u_, vpe)
                nc.vector.tensor_scalar(u_, u_, -0.5, 1.5, ALU.mult, ALU.add)
                nc.vector.tensor_mul(r0, r0, u_)
                nc.vector.tensor_mul(a_, gamma, r0)
                nc.vector.tensor_mul(c_, a_, mean)
                nc.vector.tensor_sub(c_, beta, c_)
                return prm

            def bn_apply(prm, hx2, dst):
                for k in range(NK):
                    nc.scalar.activation(
                        dst[:, k * SL:(k + 1) * SL],
                        hx2[:, k * SL:(k + 1) * SL], AFT.Identity,
                        bias=prm[:, 8 + k:9 + k], scale=prm[:, k:k + 1])

            pend_b = None
            for t in range(nsteps):
                last = (t == nsteps - 1)
                # ---- G1 (stationary u1 built at end of prev step) ----
                P1 = ppool.tile([128, H], f32, tag="P", name="P1")
                gemm(u1, "w1h", "w1l", P1)
                px1 = reduce_transpose(P1, "1")
                x1 = spool.tile([128, PW], f32, tag="x1", name="x1")
                sigmoid_into(px1, x1, "1")
                # u3 = split(x1 + hfT), chunked so G3 can start on chunk 0
                build_u_chunked(u3, x1, hfT)

                # ---- G3 (forward) ----
                P3 = ppool.tile([128, H], f32, tag="P", name="P3")
                gemm(u3, "w3h", "w3l", P3)
                px3 = reduce_transpose(P3, "3")
                hf2 = spool.tile([128, PW], f32, tag="hf2", name="hf2")
                sigmoid_into(px3, hf2, "3")
                if not last:
                    st_f = stats_of(hf2, "f")
                    outb_f = launch_ag(st_f, "f")

                # ---- previous step's backward BN -> hbT, then u2 ----
                if pend_b is not None:
                    outb_b, hb2_prev = pend_b
                    prm_b = bn_params(outb_b, gbp, bbp, "b")
                    bn_apply(prm_b, hb2_prev, hbT)
                    pend_b = None
                build_u_full(u2, hbT, x1)

                # ---- G2 (backward) ----
                P2 = ppool.tile([128, H], f32, tag="P", name="P2")
                gemm(u2, "w2h", "w2l", P2)
                px2 = reduce_transpose(P2, "2")
                hb2 = spool.tile([128, PW], f32, tag="hb2", name="hb2")
                sigmoid_into(px2, hb2, "2")

                if last:
                    o = spool.tile([128, PW], f32, tag="o")
                    nc.vector.tensor_add(o[:], hf2[:], hb2[:])
                    nc.vector.tensor_scalar_mul(o[:], o[:], 0.5)
                    nc.sync.dma_start(outp[:], o[:])
                    for nm, ap in (("x1", x1), ("hf2", hf2), ("hb2", hb2)):
                        if nm in taps:
                            nc.sync.dma_start(taps[nm][:], ap[:])
                    continue

                st_b = stats_of(hb2, "b")
                outb_b = launch_ag(st_b, "b")
                pend_b = (outb_b, hb2)

                # ---- keep-warm anchor 1: immediately after G2/E2 ----
                wp = warmp.tile([128, 512], f32, tag="wp", name="wp")
                for d in range(3):
                    nc.tensor.matmul(wp[0:64, 0:256],
                                     u1[:, d * 64:d * 64 + 64],
                                     w_sb["w1h"][:, 0:256],
                                     start=True, stop=True,
                                     skip_group_check=True)
                wscr = spool.tile([128, 8], f32, tag="wscr", name="wscr")
                nc.vector.tensor_copy(wscr[0:64, :], wp[0:64, 0:8])

                # ---- forward BN -> hfT, build u1 for next step (chunked) ----
                prm_f = bn_params(outb_f, gfp, bfp, "f")
                # keep-warm anchor 2: fires once the AG_f gather has landed,
                # bridging the HAM MID window while params are computed
                gath_f = _gath_tiles["f"]
                for d in range(3):
                    nc.tensor.matmul(wp[0:32, 256:384],
                                     gath_f[:, 0:32],
                                     gath_f[:, 0:128],
                                     start=True, stop=True,
                                     skip_group_check=True)
                nc.vector.tensor_copy(wscr[0:32, :], wp[0:32, 256:264])
                bn_apply(prm_f, hf2, hfT)
                build_u_chunked(u1, xt_sb, hfT)

    nc.compile()
    return nc


def numpy_sim(inp, nsteps):
    sig = lambda x: 1.0 / (1.0 + np.exp(-x))

    def bn(x, g, b):
        m = x.mean(0)
        xc = x - m
        v = (xc * xc).mean(0)
        return xc / np.sqrt(v + EPS) * g + b

    X = inp["inputs"]
    hf = np.zeros((S, H), np.float32)
    hb = np.zeros((S, H), np.float32)
    for t in range(nsteps):
        x1 = sig((X + hf) @ inp["W1"].T + inp["b1"])
        hb2 = sig((hb + x1) @ inp["W2"].T + inp["b2"])
        hf2 = sig((x1 + hf) @ inp["W3"].T + inp["b3"])
        out = (hf2 + hb2) * 0.5
        hf = bn(hf2, inp["gamma_f"], inp["beta_f"])
        hb = bn(hb2, inp["gamma_b"], inp["beta_b"])
    return out, x1, hf2, hb2


def make_in_maps(inp, n_cores=8):
    m = {}
    for i, wn in enumerate(("W1", "W2", "W3")):
        wh, wl = split16(np.asarray(inp[wn], np.float32))
        m[f"w{i+1}h"] = pack_w_moving(wh)
        m[f"w{i+1}l"] = pack_w_moving(wl)
    vecs = np.zeros((128, NK * 7), np.float32)
    for i, nm in enumerate(("gamma_f", "beta_f", "gamma_b", "beta_b",
                            "b1", "b2", "b3")):
        vecs[:, i * NK:(i + 1) * NK] = pack_vec(np.asarray(inp[nm], np.float32))
    m["vecs"] = vecs
    X = np.asarray(inp["inputs"], np.float32)
    maps = []
    for c in range(n_cores):
        mm = dict(m)
        mm["xt"] = pack_actT(X[c * SL:(c + 1) * SL, :])
        maps.append(mm)
    return maps


def assemble_out(results, n_cores=8):
    out = np.empty((S, H), np.float32)
    for c in range(n_cores):
        out[c * SL:(c + 1) * SL, :] = unpack_actT(results[c]["out"])
    return out


_NC_CACHE = {}


def kernel(**inputs):
    inp = {k: np.asarray(v) for k, v in inputs.items()}
    use_bias = any(np.abs(np.asarray(inp[b], np.float32)).max() > 0
                   for b in ("b1", "b2", "b3"))
    key = (S, use_bias)
    if key not in _NC_CACHE:
        _NC_CACHE[key] = build_kernel(S, use_bias=use_bias)
    nc = _NC_CACHE[key]
    maps = make_in_maps(inp)
    from concourse.bass_utils import run_bass_kernel_spmd
    res = run_bass_kernel_spmd(nc, maps, core_ids=list(range(8)))
    return assemble_out(res.results).astype(np.float32)


# revision 4
# speedup vs baseline: 1.1130x; 1.1089x over previous
"""Trainium2 Bass kernel v3 for nn_BiLSTM_58351425683848.

Self-contained: accepts FULL inputs of reference.setup_inputs(), returns the
FULL [256, 1024] output. Row-shards the sequence across 8 cores (32 rows
each); per-step BN statistics are exchanged with two staggered 8KB
AllGathers.

Arithmetic: fp16 hi/lo 3-product gemms (fp32-equivalent), but with packed
stationaries so each gemm streams each weight matrix only twice instead of
three times: the Wh stream uses stationary [uh|ul] (M=64, col strips 0-1)
and the Wl stream uses [uh] (M=32, strips 2-3, alternating by k); the
existing 4-block E-matmul reduction sums uh@Wh + ul@Wh + uh@Wl for free.
"""
import sys
sys.path.insert(0, '/opt/trn_rl_repo')
import numpy as np

S = 256
H = 1024
EPS = 1e-5
NK = 8
SL = 32
PW = NK * SL  # 256


def pack_actT(a):
    sl = a.shape[0]
    out = np.empty((128, NK * sl), a.dtype)
    for k in range(NK):
        out[:, k * sl:(k + 1) * sl] = a[:, k * 128:(k + 1) * 128].T
    return np.ascontiguousarray(out)


def unpack_actT(p, sl=SL):
    a = np.empty((sl, H), p.dtype)
    for k in range(NK):
        a[:, k * 128:(k + 1) * 128] = p[:, k * sl:(k + 1) * sl].T
    return a


def pack_w_moving(w):
    out = np.empty((128, NK * H), w.dtype)
    for k in range(NK):
        out[:, k * H:(k + 1) * H] = w[:, k * 128:(k + 1) * 128].T
    return np.ascontiguousarray(out)


def pack_vec(v):
    return np.ascontiguousarray(v.reshape(NK, 128).T)


def split16(x):
    hi = x.astype(np.float16)
    lo = (x - hi.astype(np.float32)).astype(np.float16)
    return hi, lo


def build_kernel(nsteps, n_cores=8, use_bias=False, debug_taps=()):
    import concourse.bacc as bacc
    import concourse.tile as tile
    import concourse.mybir as mybir

    f32 = mybir.dt.float32
    f16 = mybir.dt.float16
    AFT = mybir.ActivationFunctionType
    ALU = mybir.AluOpType

    nc = bacc.Bacc("TRN2", target_bir_lowering=False, debug=False,
                   num_devices=n_cores)

    xt = nc.dram_tensor("xt", [128, PW], f32, kind="ExternalInput")
    w_in = {}
    for nm in ("w1h", "w1l", "w2h", "w2l", "w3h", "w3l"):
        w_in[nm] = nc.dram_tensor(nm, [128, NK * H], f16, kind="ExternalInput")
    # vecs: gf bf gb bb b1 b2 b3 (packed [128, NK] each)
    vecs_in = nc.dram_tensor("vecs", [128, NK * 7], f32, kind="ExternalInput")
    outp = nc.dram_tensor("out", [128, PW], f32, kind="ExternalOutput")
    taps = {}
    for nm in debug_taps:
        taps[nm] = nc.dram_tensor(f"tap_{nm}", [128, PW], f32,
                                  kind="ExternalOutput")

    E_np = np.tile(np.eye(SL, dtype=np.float32), (4, 1))
    e_dram = nc.inline_tensor(E_np, name="emat")

    with tile.TileContext(nc) as tc:
        with tc.tile_pool(name="wpool", bufs=1) as wpool, \
             tc.tile_pool(name="spool", bufs=3) as spool, \
             tc.tile_pool(name="dpool", bufs=4, space="DRAM") as dpool, \
             tc.tile_pool(name="ppool", bufs=2, space="PSUM") as ppool, \
             tc.tile_pool(name="pxpool", bufs=2, space="PSUM") as pxpool, \
             tc.tile_pool(name="warmp", bufs=1, space="PSUM") as warmp:

            w_sb = {}
            for nm in w_in:
                w_sb[nm] = wpool.tile([128, NK * H], f16, tag=nm, name=nm)
                for k in range(NK):
                    nc.sync.dma_start(w_sb[nm][:, k * H:(k + 1) * H],
                                      w_in[nm][:, k * H:(k + 1) * H])
            vecs = wpool.tile([128, NK * 7], f32, tag="vecs")
            nc.sync.dma_start(vecs[:], vecs_in[:])
            gfp = vecs[:, 0 * NK:1 * NK]
            bfp = vecs[:, 1 * NK:2 * NK]
            gbp = vecs[:, 2 * NK:3 * NK]
            bbp = vecs[:, 3 * NK:4 * NK]
            bias_g = {"1": vecs[:, 4 * NK:5 * NK],
                      "2": vecs[:, 5 * NK:6 * NK],
                      "3": vecs[:, 6 * NK:7 * NK]}
            e_sb = wpool.tile([128, SL], f32, tag="emat")
            nc.sync.dma_start(e_sb[:], e_dram[:])
            xt_sb = wpool.tile([128, PW], f32, tag="xt")
            nc.sync.dma_start(xt_sb[:], xt[:])

            hfT = wpool.tile([128, PW], f32, tag="hfT")
            hbT = wpool.tile([128, PW], f32, tag="hbT")
            nc.vector.memset(hfT[:], 0.0)
            nc.vector.memset(hbT[:], 0.0)
            # packed hi/lo stationaries, per chunk k: [uh_k (32) | ul_k (32)]
            u1 = wpool.tile([128, 2 * PW], f16, tag="u1")
            u3 = wpool.tile([128, 2 * PW], f16, tag="u3")
            u2 = wpool.tile([128, 2 * PW], f16, tag="u2")

            def uh_view(u):
                return u[:].rearrange("p (k c) -> p k c", k=NK)[:, :, 0:SL]

            def ul_view(u):
                return u[:].rearrange("p (k c) -> p k c", k=NK)[:, :, SL:2 * SL]

            def v3(x):
                return x[:].rearrange("p (k s) -> p k s", k=NK)

            # init u1 = split(xt + 0)
            nc.vector.tensor_copy(uh_view(u1), v3(xt_sb))
            nc.vector.tensor_sub(ul_view(u1), v3(xt_sb), uh_view(u1))

            def build_u_full(u, a32, b32):
                """u <- hi/lo split of (a32 + b32), full-tile ops."""
                t = spool.tile([128, PW], f32, tag="uscr", name="uscr")
                nc.vector.tensor_add(t[:], a32[:], b32[:])
                nc.vector.tensor_copy(uh_view(u), v3(t))
                nc.vector.tensor_sub(ul_view(u), v3(t), uh_view(u))

            def build_u_chunked(u, a32, b32):
                for k in range(NK):
                    sl_ = slice(k * SL, (k + 1) * SL)
                    t = spool.tile([128, SL], f32, tag="ucs", name=f"ucs{k}")
                    nc.vector.tensor_add(t[:], a32[:, sl_], b32[:, sl_])
                    nc.vector.tensor_copy(u[:, 64 * k:64 * k + SL], t[:])
                    nc.vector.tensor_sub(u[:, 64 * k + SL:64 * k + 64],
                                         t[:], u[:, 64 * k:64 * k + SL])

            def gemm(u, wh, wl, P):
                """P[128,1024] <- 3-product fp16 gemm with packed stationaries.

                Adjacent MMs alternate BOTH col strip and PSUM bank so their
                fill/drain phases overlap (pairing needs distinct banks)."""
                whs, wls = w_sb[wh], w_sb[wl]
                for k in range(NK):
                    for nh in range(2):
                        nc.tensor.matmul(
                            P[0:64, 512 * nh:512 * (nh + 1)],
                            u[:, 64 * k:64 * k + 64],
                            whs[:, k * H + 512 * nh:k * H + 512 * (nh + 1)],
                            start=(k == 0), stop=(k == NK - 1),
                            tile_position=(0, 0),
                        )
                for k in range(NK):
                    p = k % 2
                    for nh in range(2):
                        nc.tensor.matmul(
                            P[64 + 32 * p:96 + 32 * p, 512 * nh:512 * (nh + 1)],
                            u[:, 64 * k:64 * k + SL],
                            wls[:, k * H + 512 * nh:k * H + 512 * (nh + 1)],
                            start=(k == p), stop=(k == NK - 2 + p),
                            tile_position=(0, 64 + 32 * p),
                        )

            def reduce_transpose(P, tag):
                Ssb = spool.tile([128, H], f32, tag="Ssb", name="Ssb" + tag)
                nc.scalar.activation(Ssb[:, 0:512], P[:, 0:512], AFT.Copy)
                nc.vector.tensor_copy(Ssb[:, 512:1024], P[:, 512:1024])
                px = pxpool.tile([128, PW], f32, tag="px", name="px" + tag)
                for j in range(NK):
                    nc.tensor.matmul(
                        px[:, j * SL:(j + 1) * SL],
                        Ssb[:, j * 128:(j + 1) * 128],
                        e_sb[:],
                        start=True, stop=True,
                    )
                return px

            def sigmoid_into(px, dst, which):
                if use_bias:
                    for k in range(NK):
                        nc.scalar.activation(
                            dst[:, k * SL:(k + 1) * SL],
                            px[:, k * SL:(k + 1) * SL], AFT.Sigmoid,
                            bias=bias_g[which][:, k:k + 1])
                else:
                    nc.scalar.activation(dst[:], px[:], AFT.Sigmoid)

            def stats_of(hx2, tag):
                st = spool.tile([128, 16], f32, tag="st" + tag, name="st" + tag)
                nc.vector.tensor_reduce(st[:, 0:8], v3(hx2),
                                        axis=mybir.AxisListType.X, op=ALU.add)
                sq_ = spool.tile([128, PW], f32, tag="sqscr", name="sq" + tag)
                nc.scalar.activation(sq_[:], hx2[:], AFT.Square)
                nc.vector.tensor_reduce(st[:, 8:16], v3(sq_),
                                        axis=mybir.AxisListType.X, op=ALU.add)
                return st

            def launch_ag(st, tag):
                inb = dpool.tile([128, 16], f32, tag="agi" + tag,
                                 name="agi" + tag)
                outb = dpool.tile([128 * n_cores, 16], f32, tag="ago" + tag,
                                  name="ago" + tag)
                nc.sync.dma_start(inb[:], st[:])
                nc.gpsimd.collective_compute(
                    "AllGather", ALU.bypass,
                    replica_groups=[list(range(n_cores))],
                    ins=[inb.opt()], outs=[outb.opt()],
                )
                return outb

            _gath_tiles = {}

            def bn_params(outb, gamma, beta, tag):
                gath = spool.tile([128, n_cores * 16], f32, tag="gath" + tag,
                                  name="gath" + tag)
                _gath_tiles[tag] = gath
                nc.sync.dma_start(
                    gath[:].rearrange("p (r c) -> p r c", r=n_cores),
                    outb[:].rearrange("(r p) c -> p r c", p=128))
                tot = spool.tile([128, 16], f32, tag="tot" + tag,
                                 name="tot" + tag)
                nc.vector.tensor_reduce(
                    tot[:], gath[:].rearrange("p (r c) -> p c r", r=n_cores),
                    axis=mybir.AxisListType.X, op=ALU.add)
                prm = spool.tile([128, 48], f32, tag="prm" + tag,
                                 name="prm" + tag)
                a_ = prm[:, 0:8]
                c_ = prm[:, 8:16]
                mean = prm[:, 16:24]
                vpe = prm[:, 24:32]
                u_ = prm[:, 32:40]
                r0 = prm[:, 40:48]
                nc.vector.tensor_scalar_mul(mean, tot[:, 0:8], 1.0 / S)
                nc.vector.tensor_mul(u_, mean, mean)
                nc.vector.tensor_scalar(vpe, tot[:, 8:16], 1.0 / S, EPS,
                                        ALU.mult, ALU.add)
                nc.vector.tensor_sub(vpe, vpe, u_)  # var + eps
                sq = spool.tile([128, 8], f32, tag="sqv" + tag,
                                name="sqv" + tag)
                nc.scalar.activation(sq[:], vpe, AFT.Sqrt)
                nc.vector.reciprocal(r0, sq[:])
                nc.vector.tensor_mul(u_, r0, r0)
                nc.vector.tensor_mul(u_, u_, vpe)
                nc.vector.tensor_scalar(u_, u_, -0.5, 1.5, ALU.mult, ALU.add)
                nc.vector.tensor_mul(r0, r0, u_)
                nc.vector.tensor_mul(a_, gamma, r0)
                nc.vector.tensor_mul(c_, a_, mean)
                nc.vector.tensor_sub(c_, beta, c_)
                return prm

            def bn_apply(prm, hx2, dst):
                for k in range(NK):
                    nc.scalar.activation(
                        dst[:, k * SL:(k + 1) * SL],
                        hx2[:, k * SL:(k + 1) * SL], AFT.Identity,
                        bias=prm[:, 8 + k:9 + k], scale=prm[:, k:k + 1])

            pend_b = None
            for t in range(nsteps):
                last = (t == nsteps - 1)
                # ---- G1 (stationary u1 built at end of prev step) ----
                P1 = ppool.tile([128, H], f32, tag="P", name="P1")
                gemm(u1, "w1h", "w1l", P1)
                px1 = reduce_transpose(P1, "1")
                x1 = spool.tile([128, PW], f32, tag="x1", name="x1")
                sigmoid_into(px1, x1, "1")
                # u3 = split(x1 + hfT), chunked so G3 can start on chunk 0
                build_u_chunked(u3, x1, hfT)

                # ---- G3 (forward) ----
                P3 = ppool.tile([128, H], f32, tag="P", name="P3")
                gemm(u3, "w3h", "w3l", P3)
                px3 = reduce_transpose(P3, "3")
                hf2 = spool.tile([128, PW], f32, tag="hf2", name="hf2")
                sigmoid_into(px3, hf2, "3")
                if not last:
                    st_f = stats_of(hf2, "f")
                    outb_f = launch_ag(st_f, "f")

                # ---- previous step's backward BN -> hbT, then u2 ----
                if pend_b is not None:
                    outb_b, hb2_prev = pend_b
                    prm_b = bn_params(outb_b, gbp, bbp, "b")
                    bn_apply(prm_b, hb2_prev, hbT)
                    pend_b = None
                build_u_full(u2, hbT, x1)

                # ---- G2 (backward) ----
                P2 = ppool.tile([128, H], f32, tag="P", name="P2")
                gemm(u2, "w2h", "w2l", P2)
                px2 = reduce_transpose(P2, "2")
                hb2 = spool.tile([128, PW], f32, tag="hb2", name="hb2")
                sigmoid_into(px2, hb2, "2")

                if last:
                    o = spool.tile([128, PW], f32, tag="o")
                    nc.vector.tensor_add(o[:], hf2[:], hb2[:])
                    nc.vector.tensor_scalar_mul(o[:], o[:], 0.5)
                    nc.sync.dma_start(outp[:], o[:])
                    for nm, ap in (("x1", x1), ("hf2", hf2), ("hb2", hb2)):
                        if nm in taps:
                            nc.sync.dma_start(taps[nm][:], ap[:])
                    continue

                st_b = stats_of(hb2, "b")
                outb_b = launch_ag(st_b, "b")
                pend_b = (outb_b, hb2)

                # ---- keep-warm anchor 1: immediately after G2/E2 ----
                wp = warmp.tile([128, 512], f32, tag="wp", name="wp")
                for d in range(6):
                    nc.tensor.matmul(wp[0:64, 0:256],
                                     u1[:, d * 64:d * 64 + 64],
                                     w_sb["w1h"][:, d * 512:d * 512 + 256],
                                     start=True, stop=True,
                                     skip_group_check=True)
                wscr = spool.tile([128, 8], f32, tag="wscr", name="wscr")
                nc.vector.tensor_copy(wscr[0:64, :], wp[0:64, 0:8])

                # ---- forward BN -> hfT, build u1 for next step (chunked) ----
                prm_f = bn_params(outb_f, gfp, bfp, "f")
                # keep-warm anchor 2: fires once the AG_f gather has landed,
                # bridging the HAM MID window while params are computed
                gath_f = _gath_tiles["f"]
                for d in range(4):
                    nc.tensor.matmul(wp[0:32, 256:384],
                                     gath_f[:, 0:32],
                                     gath_f[:, 0:128],
                                     start=True, stop=True,
                                     skip_group_check=True)
                nc.vector.tensor_copy(wscr[0:32, :], wp[0:32, 256:264])
                bn_apply(prm_f, hf2, hfT)
                # keep-warm anchor 3: fires as the applies complete, bridging
                # the params/apply/u1-build stretch right up to next G1
                for d in range(3):
                    nc.tensor.matmul(wp[0:32, 384:512],
                                     hfT[:, 0:32],
                                     hfT[:, 0:128],
                                     start=True, stop=True,
                                     skip_group_check=True)
                nc.vector.tensor_copy(wscr[0:32, :], wp[0:32, 384:392])
                build_u_chunked(u1, xt_sb, hfT)

    nc.compile()
    return nc


def numpy_sim(inp, nsteps):
    sig = lambda x: 1.0 / (1.0 + np.exp(-x))

    def bn(x, g, b):
        m = x.mean(0)
        xc = x - m
        v = (xc * xc).mean(0)
        return xc / np.sqrt(v + EPS) * g + b

    X = inp["inputs"]
    hf = np.zeros((S, H), np.float32)
    hb = np.zeros((S, H), np.float32)
    for t in range(nsteps):
        x1 = sig((X + hf) @ inp["W1"].T + inp["b1"])
        hb2 = sig((hb + x1) @ inp["W2"].T + inp["b2"])
        hf2 = sig((x1 + hf) @ inp["W3"].T + inp["b3"])
        out = (hf2 + hb2) * 0.5
        hf = bn(hf2, inp["gamma_f"], inp["beta_f"])
        hb = bn(hb2, inp["gamma_b"], inp["beta_b"])
    return out, x1, hf2, hb2


def make_in_maps(inp, n_cores=8):
    m = {}
    for i, wn in enumerate(("W1", "W2", "W3")):
        wh, wl = split16(np.asarray(inp[wn], np.float32))
        m[f"w{i+1}h"] = pack_w_moving(wh)
        m[f"w{i+1}l"] = pack_w_moving(wl)
    vecs = np.zeros((128, NK * 7), np.float32)
    for i, nm in enumerate(("gamma_f", "beta_f", "gamma_b", "beta_b",
                            "b1", "b2", "b3")):
        vecs[:, i * NK:(i + 1) * NK] = pack_vec(np.asarray(inp[nm], np.float32))
    m["vecs"] = vecs
    X = np.asarray(inp["inputs"], np.float32)
    maps = []
    for c in range(n_cores):
        mm = dict(m)
        mm["xt"] = pack_actT(X[c * SL:(c + 1) * SL, :])
        maps.append(mm)
    return maps


def assemble_out(results, n_cores=8):
    out = np.empty((S, H), np.float32)
    for c in range(n_cores):
        out[c * SL:(c + 1) * SL, :] = unpack_actT(results[c]["out"])
    return out


_NC_CACHE = {}


def kernel(**inputs):
    inp = {k: np.asarray(v) for k, v in inputs.items()}
    use_bias = any(np.abs(np.asarray(inp[b], np.float32)).max() > 0
                   for b in ("b1", "b2", "b3"))
    key = (S, use_bias)
    if key not in _NC_CACHE:
        _NC_CACHE[key] = build_kernel(S, use_bias=use_bias)
    nc = _NC_CACHE[key]
    maps = make_in_maps(inp)
    from concourse.bass_utils import run_bass_kernel_spmd
    res = run_bass_kernel_spmd(nc, maps, core_ids=list(range(8)))
    return assemble_out(res.results).astype(np.float32)
